# revision 5
# baseline (speedup 1.0000x reference)
"""Trainium2 Bass kernel for nn_CPLinear (CP-decomposed QKV projection with RoPE).

Computes, for x:(2,4096,2048) and CP-factor weights:
    A_t = x @ W_A_t  (per-token head coefficients),  B_t = x @ W_B_t (shared bases)
    q = einsum('bshr,bsrd->bshd', A_q, rope(B_q)) / 12
    k = A_k * rope(B_k)   (rank-1)
    v = A_v * B_v         (rank-1)

Strategy (8 cores, data-parallel over the 8192 tokens, 1024 tokens/core):
  - All 6 projections fused into one [2048 x 2016] bf16 matmul (PE), with the
    1/12 scale and (h,r)->(r,h) reorder folded into W_A_q host-side.
  - Weights stream in 16 per-k-tile chunks and x in 8 per-token-tile chunks so
    the PE starts ~3us in; a k-outer warm-up phase accumulates tile0 fully plus
    the 480-col A-blocks of tiles 1-2 while the weight load is in flight.
  - Steady state decouples the 480-col A-series (2 slots ahead) from the
    1536-col B_q series, so PSUM bank reuse never stalls the PE.
  - The per-token rank-12 contraction for q runs on the PE as a block-diagonal
    matmul (8 tokens x 12 r = 96 K, 8x16=128 M, 128 N); its operands are built
    by a DRAM-bounce scatter. Those 16 matmuls per tile are interleaved into
    the NEXT tile's projection stream so their LDWEIGHTS hide under wide MMs.
  - k/v are per-partition-scalar broadcasts (DVE); outputs written bf16,
    widened to fp32 on the host.
"""

import sys

for _p in ("/opt/trn_rl_repo",):
    if _p not in sys.path:
        sys.path.insert(0, _p)

import numpy as np
import ml_dtypes

BF16 = ml_dtypes.bfloat16

SH = 1024          # tokens per core
H = 2048           # hidden
KT = H // 128      # 16 k-tiles
NT = SH // 128     # 8 token tiles per core
NOUT = 2016        # fused projection output width
NH, HD, RQ = 16, 128, 12

_CACHE = {}


def make_nc():
    import concourse.bacc as bacc
    from concourse import mybir

    dt = mybir.dt

    nc = bacc.Bacc(
        "TRN2",
        target_bir_lowering=False,
        debug=False,
        enable_asserts=False,
        num_devices=8,
    )

    x_d = nc.dram_tensor("x", (H, SH), dt.bfloat16, kind="ExternalInput")  # pre-transposed host-side
    w_d = nc.dram_tensor("w", (KT, 128, NOUT), dt.bfloat16, kind="ExternalInput")
    cos_d = nc.dram_tensor("cosr", (SH, 64), dt.bfloat16, kind="ExternalInput")
    sin_d = nc.dram_tensor("sinr", (SH, 64), dt.bfloat16, kind="ExternalInput")
    q_d = nc.dram_tensor("q", (SH, NH, HD), dt.bfloat16, kind="ExternalOutput")
    k_d = nc.dram_tensor("k", (SH, NH * HD), dt.bfloat16, kind="ExternalOutput")
    v_d = nc.dram_tensor("v", (SH, NH * HD), dt.bfloat16, kind="ExternalOutput")
    return nc, (x_d, w_d, cos_d, sin_d, q_d, k_d, v_d)


def build_body(nc, tc, tensors):
    from contextlib import ExitStack

    from concourse import mybir

    dt = mybir.dt
    x_d, w_d, cos_d, sin_d, q_d, k_d, v_d = tensors

    with ExitStack() as ctx:
        P = ctx.enter_context
        const_pool = P(tc.tile_pool(name="const", bufs=1))
        w_sb = const_pool.tile([128, KT * NOUT], dt.bfloat16, tag="w_sb")
        cos_sb = const_pool.tile([128, NT * 64], dt.bfloat16, tag="cos_sb")
        sin_sb = const_pool.tile([128, NT * 64], dt.bfloat16, tag="sin_sb")
        xT = const_pool.tile([128, KT * SH], dt.bfloat16, tag="xT")
        lhs_bufs = [
            const_pool.tile([128, 2048], dt.bfloat16, tag=f"lhs{i}", name=f"lhs{i}")
            for i in range(3)
        ]
        bdr_bufs = [
            const_pool.tile([128, 2048], dt.bfloat16, tag=f"bdr{i}", name=f"bdr{i}")
            for i in range(3)
        ]

        w_v = w_sb[:].rearrange("p (k n) -> p k n", k=KT)
        wd_v = w_d[:].rearrange("k p n -> p k n")
        x_v = xT[:].rearrange("p (k t) -> p k t", k=KT)
        xd_v = x_d[:].rearrange("(k p) t -> p k t", p=128)

        # ---- input DMA prologue ----
        # gpsimd queue: rope tables + x tiles 1-2 (needed in the warm-up phase)
        nc.gpsimd.dma_start(
            out=cos_sb[:].rearrange("p (t n) -> p t n", t=NT),
            in_=cos_d[:].rearrange("(t p) n -> p t n", p=128),
        )
        nc.gpsimd.dma_start(
            out=sin_sb[:].rearrange("p (t n) -> p t n", t=NT),
            in_=sin_d[:].rearrange("(t p) n -> p t n", p=128),
        )

        def dma_x(j, eng):
            sl = slice(j * 128, (j + 1) * 128)
            eng.dma_start(out=x_v[:, :, sl], in_=xd_v[:, :, sl])

        def dma_w(k, eng):
            eng.dma_start(out=w_v[:, k], in_=wd_v[:, k])

        dma_x(1, nc.gpsimd)
        dma_x(2, nc.gpsimd)
        # x0 ahead of the odd w chunks on sync; w chunks alternate two queues
        # in k order so chunk k lands ~k*1.3us in. Deferred x tiles 3-7 sit
        # BEHIND the w chunks on the same queues so they don't steal HBM
        # bandwidth from the critical weight load.
        dma_x(0, nc.sync)
        for k in range(KT):
            dma_w(k, nc.scalar if k % 2 == 0 else nc.sync)
        for j in range(3, NT):
            dma_x(j, nc.scalar if j % 2 == 1 else nc.sync)

        for tl in lhs_bufs + bdr_bufs:
            nc.gpsimd.memset(tl[:], 0.0)

        psa_pool = P(tc.tile_pool(name="psa", bufs=3, space="PSUM"))
        psb_pool = P(tc.tile_pool(name="psb", bufs=1, space="PSUM"))
        psq_pool = P(tc.tile_pool(name="psq", bufs=2, space="PSUM"))
        bq_pool = P(tc.tile_pool(name="bq", bufs=2))
        bqr_pool = P(tc.tile_pool(name="bqr", bufs=3))
        tmp_pool = P(tc.tile_pool(name="tmp", bufs=2))
        small_pool = P(tc.tile_pool(name="small", bufs=3))
        out_pool = P(tc.tile_pool(name="outs", bufs=2))
        dram_pool = P(tc.tile_pool(name="scr", bufs=4, space="DRAM"))

        # per-tile state
        ps_a = {}
        ps_b = {}
        bqr = {}
        scr = {}
        qsb = {}

        def a_mm(i, k):
            if i not in ps_a:
                ps_a[i] = psa_pool.tile(
                    [128, 512], dt.float32, tag="ps_a", name=f"ps_a{i}"
                )
            lh = xT[:, k * SH + i * 128 : k * SH + i * 128 + 128]
            nc.tensor.matmul(
                ps_a[i][:, 0:480],
                lh,
                w_sb[:, k * NOUT : k * NOUT + 480],
                start=(k == 0),
                stop=(k == KT - 1),
            )

        def b_mm(i, k):
            if i not in ps_b:
                ps_b[i] = psb_pool.tile(
                    [128, 1536], dt.float32, tag="ps_b", name=f"ps_b{i}"
                )
            lh = xT[:, k * SH + i * 128 : k * SH + i * 128 + 128]
            wb = k * NOUT + 480
            for c in range(3):
                nc.tensor.matmul(
                    ps_b[i][:, c * 512 : (c + 1) * 512],
                    lh,
                    w_sb[:, wb + c * 512 : wb + (c + 1) * 512],
                    start=(k == 0),
                    stop=(k == KT - 1),
                )

        def evict_a(i):
            """A-block eviction for tile i: A' + per-token scalars, rope(B_k),
            k/v outputs, and the early (A'-side) half of the scatter."""
            t0 = i * 128
            pa = ps_a.pop(i)
            bqr[i] = bqr_pool.tile(
                [128, 1728], dt.bfloat16, tag="bqr", name=f"bqr{i}"
            )
            ak_sb = small_pool.tile([128, 16], dt.bfloat16, tag="ak_sb")
            av_sb = small_pool.tile([128, 16], dt.bfloat16, tag="av_sb")
            bk_sb = small_pool.tile([128, 128], dt.bfloat16, tag="bk_sb")
            bkr_sb = small_pool.tile([128, 128], dt.bfloat16, tag="bkr_sb")
            bv_sb = small_pool.tile([128, 128], dt.bfloat16, tag="bv_sb")
            nc.scalar.copy(bqr[i][:, 1536:1728], pa[:, 0:192])
            nc.scalar.copy(ak_sb[:], pa[:, 192:208])
            nc.scalar.copy(av_sb[:], pa[:, 208:224])
            nc.scalar.copy(bk_sb[:], pa[:, 224:352])
            nc.scalar.copy(bv_sb[:], pa[:, 352:480])

            # rope B_k (DVE)
            tk_a = small_pool.tile([128, 64], dt.bfloat16, tag="tk_a")
            tk_b = small_pool.tile([128, 64], dt.bfloat16, tag="tk_b")
            bkv = bk_sb[:].rearrange("p (two d) -> p two d", two=2)
            bkrv = bkr_sb[:].rearrange("p (two d) -> p two d", two=2)
            cos_k = cos_sb[:, i * 64 : (i + 1) * 64]
            sin_k = sin_sb[:, i * 64 : (i + 1) * 64]
            nc.vector.tensor_mul(tk_a[:], bkv[:, 0], cos_k)
            nc.vector.tensor_mul(tk_b[:], bkv[:, 1], sin_k)
            nc.vector.tensor_sub(bkrv[:, 0], tk_a[:], tk_b[:])
            nc.vector.tensor_mul(tk_a[:], bkv[:, 1], cos_k)
            nc.vector.tensor_mul(tk_b[:], bkv[:, 0], sin_k)
            nc.vector.tensor_add(bkrv[:, 1], tk_a[:], tk_b[:])

            # k/v rank-1 broadcasts + output DMAs
            ksb = out_pool.tile([128, 2048], dt.bfloat16, tag="ksb")
            vsb = out_pool.tile([128, 2048], dt.bfloat16, tag="vsb")
            nc.vector.tensor_mul(
                ksb[:].rearrange("p (h d) -> p h d", h=NH),
                bkr_sb[:].unsqueeze(1).broadcast_to([128, NH, 128]),
                ak_sb[:].unsqueeze(2).broadcast_to([128, NH, 128]),
            )
            nc.vector.tensor_mul(
                vsb[:].rearrange("p (h d) -> p h d", h=NH),
                bv_sb[:].unsqueeze(1).broadcast_to([128, NH, 128]),
                av_sb[:].unsqueeze(2).broadcast_to([128, NH, 128]),
            )
            nc.sync.dma_start(out=k_d[t0 : t0 + 128, :], in_=ksb[:])
            nc.scalar.dma_start(out=v_d[t0 : t0 + 128, :], in_=vsb[:])

            # A'-side scatter bounce: write early, read back block-diagonal
            scr[i] = dram_pool.tile(
                [128, 1728], dt.bfloat16, tag="scr", name=f"scr{i}"
            )
            nc.sync.dma_start(out=scr[i][:, 1536:1728], in_=bqr[i][:, 1536:1728])
            lhs = lhs_bufs[i % 3]
            sa_v = scr[i][:, 1536:1728].rearrange(
                "(g t) (r h) -> t r g h", t=8, r=RQ
            )
            l_v = lhs[0:96, :].rearrange("(t r) (g c) -> t r g c", t=8, g=16)
            for t in range(8):
                nc.gpsimd.dma_start(
                    out=l_v[t][:, :, t * 16 : (t + 1) * 16], in_=sa_v[t]
                )

        def evict_b(i):
            """B_q eviction + rope + B-side scatter bounce for tile i."""
            pb = ps_b.pop(i)
            bq_sb = bq_pool.tile([128, 1536], dt.bfloat16, tag="bq_sb")
            nc.scalar.copy(bq_sb[:], pb[:])

            t_a = tmp_pool.tile([128, 768], dt.bfloat16, tag="t_a")
            t_b = tmp_pool.tile([128, 768], dt.bfloat16, tag="t_b")
            bqv = bq_sb[:].rearrange("p (r two d) -> p r two d", r=RQ, two=2)
            bqrv = bqr[i][:, 0:1536].rearrange(
                "p (r two d) -> p r two d", r=RQ, two=2
            )
            cos_t = (
                cos_sb[:, i * 64 : (i + 1) * 64]
                .unsqueeze(1)
                .broadcast_to([128, RQ, 64])
            )
            sin_t = (
                sin_sb[:, i * 64 : (i + 1) * 64]
                .unsqueeze(1)
                .broadcast_to([128, RQ, 64])
            )
            tav = t_a[:].rearrange("p (r d) -> p r d", r=RQ)
            tbv = t_b[:].rearrange("p (r d) -> p r d", r=RQ)
            p_lo = bqv[:, :, 0]
            p_hi = bqv[:, :, 1]
            nc.vector.tensor_mul(tav, p_lo, cos_t)
            nc.vector.tensor_mul(tbv, p_hi, sin_t)
            nc.vector.tensor_sub(bqrv[:, :, 0], tav, tbv)
            nc.vector.tensor_mul(tav, p_hi, cos_t)
            nc.vector.tensor_mul(tbv, p_lo, sin_t)
            nc.vector.tensor_add(bqrv[:, :, 1], tav, tbv)

            nc.sync.dma_start(out=scr[i][:, 0:1536], in_=bqr[i][:, 0:1536])
            bdr = bdr_bufs[i % 3]
            sb_v = scr[i][:, 0:1536].rearrange("(g t) (r d) -> t r g d", t=8, r=RQ)
            d_v = bdr[0:96, :].rearrange("(t r) (g d) -> t r g d", t=8, g=16)
            for t in range(8):
                eng = nc.sync if t % 2 == 0 else nc.scalar
                eng.dma_start(out=d_v[t], in_=sb_v[t])

        def bd_group(i, j):
            """Block-diagonal q matmul group j (4 of 16 per tile) + DVE evict."""
            if i not in qsb:
                qsb[i] = out_pool.tile(
                    [128, 2048], dt.bfloat16, tag="qsb", name=f"qsb{i}"
                )
            lhs = lhs_bufs[i % 3]
            bdr = bdr_bufs[i % 3]
            qp = psq_pool.tile([128, 512], dt.float32, tag="qp", name=f"qp{i}_{j}")
            for u in range(4):
                g = j * 4 + u
                nc.tensor.matmul(
                    qp[:, u * 128 : (u + 1) * 128],
                    lhs[0:96, g * 128 : (g + 1) * 128],
                    bdr[0:96, g * 128 : (g + 1) * 128],
                    start=True,
                    stop=True,
                )
            nc.vector.tensor_copy(qsb[i][:, j * 512 : (j + 1) * 512], qp[:])

        def q_out(i):
            t0 = i * 128
            nc.gpsimd.dma_start(
                out=q_d[t0 : t0 + 128].rearrange("(g t) h d -> (t h) g d", g=16),
                in_=qsb.pop(i)[:].rearrange("p (g d) -> p g d", g=16),
            )

        # ---- phase A: k-outer accumulation while weights stream in ----
        for k in range(KT):
            a_mm(0, k)
            b_mm(0, k)
            a_mm(1, k)
            a_mm(2, k)
        evict_a(0)
        evict_b(0)
        evict_a(1)
        evict_a(2)

        # ---- steady-state slots ----
        # slot s: a-series of tile s+2, b-series of tile s, BD matmuls of
        # tile s-1 interleaved into the back half, then evicts.
        BD_KS = (9, 11, 13, 15)
        for s in range(1, 8):
            for k in range(KT):
                if s <= 5:
                    a_mm(s + 2, k)
                if k >= 4:
                    b_mm(s, k - 4)
                if k in BD_KS:
                    bd_group(s - 1, BD_KS.index(k))
            for k in range(KT - 4, KT):
                b_mm(s, k)
            if s <= 5:
                evict_a(s + 2)
            evict_b(s)
            q_out(s - 1)

        # ---- tail: BD + q output of the last tile ----
        for j in range(4):
            bd_group(7, j)
        q_out(7)


def build_program():
    import concourse.tile as tile

    nc, tensors = make_nc()
    with tile.TileContext(nc) as tc:
        build_body(nc, tc, tensors)
    nc.compile()
    return nc


def _get_program():
    if "nc" not in _CACHE:
        _CACHE["nc"] = build_program()
    return _CACHE["nc"]


def make_in_maps(x, W_A_q, W_B_q, W_A_k, W_B_k, W_A_v, W_B_v):
    """Shard + preprocess full inputs into per-core input maps."""
    x = np.asarray(x)
    B, S, Hh = x.shape
    x2 = np.ascontiguousarray(x.reshape(B * S, Hh))

    # fold the 1/RQ scale and the (h,r)->(r,h) column reorder into W_A_q
    WAq = np.asarray(W_A_q).reshape(Hh, NH, RQ).transpose(0, 2, 1).reshape(
        Hh, NH * RQ
    ) / np.float32(RQ)
    Wall = np.concatenate(
        [
            WAq,
            np.asarray(W_A_k),
            np.asarray(W_A_v),
            np.asarray(W_B_k),
            np.asarray(W_B_v),
            np.asarray(W_B_q),
        ],
        axis=1,
    )
    assert Wall.shape == (Hh, NOUT)
    Wt = np.ascontiguousarray(Wall.reshape(KT, 128, NOUT)).astype(BF16)

    inv = 1.0 / (10000.0 ** (np.arange(0, HD, 2, dtype=np.float32) / HD))
    ang = np.arange(S, dtype=np.float32)[:, None] * inv[None, :]
    cos_rep = np.ascontiguousarray(np.cos(ang)).astype(BF16)
    sin_rep = np.ascontiguousarray(np.sin(ang)).astype(BF16)

    in_maps = []
    for i in range(8):
        tok0 = i * SH
        pos = np.arange(tok0, tok0 + SH) % S
        in_maps.append(
            {
                # pre-transposed (hidden, tokens) so on-chip loads are plain
                "x": np.ascontiguousarray(x2[tok0 : tok0 + SH].T).astype(BF16),
                "w": Wt,
                "cosr": np.ascontiguousarray(cos_rep[pos]),
                "sinr": np.ascontiguousarray(sin_rep[pos]),
            }
        )
    return in_maps, (B, S)


def assemble_outputs(results, B, S):
    q = np.concatenate(
        [results[i]["q"].astype(np.float32) for i in range(8)], axis=0
    ).reshape(B, S, NH, HD)
    k = np.concatenate(
        [results[i]["k"].astype(np.float32) for i in range(8)], axis=0
    ).reshape(B, S, NH, HD)
    v = np.concatenate(
        [results[i]["v"].astype(np.float32) for i in range(8)], axis=0
    ).reshape(B, S, NH, HD)
    return q, k, v


def kernel(x, W_A_q, W_B_q, W_A_k, W_B_k, W_A_v, W_B_v):
    from concourse.bass_utils import run_bass_kernel_spmd

    nc = _get_program()
    in_maps, (B, S) = make_in_maps(x, W_A_q, W_B_q, W_A_k, W_B_k, W_A_v, W_B_v)
    res = run_bass_kernel_spmd(nc, in_maps, list(range(8))).results
    return assemble_outputs(res, B, S)
